# revision 1
# baseline (speedup 1.0000x reference)
"""Contrastive loss on 8 TRN2 cores — v3 (fp8 RS, k-outer gram, lean front-end).

v2 -> v3: S=16 and fp8e4 sim ReduceScatters (halved collective bytes; with
S=16 even the fully-summed diagonal (256) stays in e4m3 range); squares split
ACT/DVE only (GpSimd+DVE concurrently thrash SBUF: 2.5x degradation measured);
normalize DVE-only, overlapped with the first gram pass via a k-outer loop
over 8 rotating 1-bank PSUM column tiles; ssq PSUM shares the same slot pool;
ch1 loss work (ex1/colsums/diag) issued before RS2 completes; positives taken
as ln(exp-diag) of the already-exponentiated pos tile.
"""

import numpy as np
import ml_dtypes

import concourse.bacc as bacc
import concourse.mybir as mybir
import concourse.tile as tile
from concourse import bass_utils

F32 = mybir.dt.float32
I16 = mybir.dt.int16
I32 = mybir.dt.int32
F8E5 = mybir.dt.float8e5
BF16 = mybir.dt.bfloat16
FP8 = mybir.dt.float8e4
AF = mybir.ActivationFunctionType
ALU = mybir.AluOpType
PM = mybir.MatmulPerfMode

B = 1024
R = 2 * B
NCORES = 8
KT = 8
S = 16.0
INV_T_S2 = 2.0 / (S * S)
CH1 = 1024
CH2 = 1152
PK = 64.0          # int16 lane scale
PKB = 16.0 * PK    # lane offset*scale
DEC_SCALE = INV_T_S2 / PK
DEC_BIAS = -128.0 * INV_T_S2

_CACHE = {}


def _pieces(c, lo, hi):
    """Segment pieces of ch2-local cols [lo,hi) for pair c.

    Returns (psum_off, sim_col, width, tile_row) pieces, 512-split."""
    wa2 = 1024 - 128 * c
    tb = 15 - c
    segs = []
    a0, a1 = max(lo, 0), min(hi, wa2)
    if a1 > a0:
        segs.append((a0 - lo, 128 * c + CH1 + a0, a1 - a0, c))
    b0, b1 = max(lo, wa2), min(hi, CH2)
    if b1 > b0:
        segs.append((b0 - lo, 128 * tb + (b0 - wa2), b1 - b0, tb))
    out = []
    for po, sc, w, tr in segs:
        off = 0
        while off < w:
            ww = min(512, w - off)
            out.append((po + off, sc + off, ww, tr))
            off += ww
    return out


def _build_nc():
    if "nc" in _CACHE:
        return _CACHE["nc"]
    nc = bacc.Bacc("TRN2", target_bir_lowering=False, debug=False,
                   num_devices=NCORES)

    x = nc.dram_tensor("x", [KT, 128, 2 * R], FP8, kind="ExternalInput")
    sel = nc.dram_tensor("sel", [128, 256], FP8, kind="ExternalInput")
    eye = nc.dram_tensor("eye", [128, 128], BF16, kind="ExternalInput")
    msk = nc.dram_tensor("msk", [2, 128, CH2], BF16, kind="ExternalInput")
    rt = nc.dram_tensor("rt", [16, 16], BF16, kind="ExternalInput")
    cm = nc.dram_tensor("cm", [2, 128, 16], F32, kind="ExternalInput")
    y = nc.dram_tensor("y", [1, 1], F32, kind="ExternalOutput")

    cc1_in = nc.dram_tensor("cc1_in", [1024, CH1], F8E5)
    cc1_out = nc.dram_tensor("cc1_out", [128, CH1], F8E5)
    cc2_in = nc.dram_tensor("cc2_in", [1024, CH2], F8E5)
    cc2_out = nc.dram_tensor("cc2_out", [128, CH2], F8E5)
    ccd_in = nc.dram_tensor("ccd_in", [128, 16], F32)
    ccd_out = nc.dram_tensor("ccd_out", [16, 16], F32)
    wu_in = nc.dram_tensor("wu_in", [128, 2], F32)
    wu_out = nc.dram_tensor("wu_out", [16, 2], F32)
    grp = [list(range(NCORES))]

    with tile.TileContext(nc) as tc:
        with tc.tile_pool(name="x8", bufs=KT) as px8, \
             tc.tile_pool(name="sq", bufs=KT) as psq, \
             tc.tile_pool(name="pers", bufs=1) as pers, \
             tc.tile_pool(name="simsb", bufs=8) as psim, \
             tc.tile_pool(name="loss", bufs=1) as plo, \
             tc.tile_pool(name="sm", bufs=1) as psm, \
             tc.tile_pool(name="ps", bufs=8, space="PSUM") as pps:

            # ACT table preloads
            junk = pers.tile([128, 16], F32, tag="junk")
            nc.vector.memset(junk[:], 1.0)
            junk2 = pers.tile([128, 16], F32, tag="junk2")
            nc.scalar.activation(junk2[:], junk[:], AF.Abs_reciprocal_sqrt)
            junk3 = pers.tile([128, 16], F32, tag="junk3")
            nc.scalar.activation(junk3[:], junk[:], AF.Exp)
            junk4 = pers.tile([128, 16], F32, tag="junk4")
            nc.scalar.activation(junk4[:], junk[:], AF.Ln)
            junk5 = pers.tile([128, 16], F32, tag="junk5")
            nc.scalar.activation(junk5[:], junk[:], AF.Square)

            # ---- load x first (everything gates on it) ----
            xb = []
            for k in range(KT):
                t = px8.tile([128, 2 * R], FP8, tag="x8")
                nc.sync.dma_start(t[0:64, :], x[k, 0:64, :])
                nc.sync.dma_start(t[64:128, :], x[k, 64:128, :])
                xb.append(t)
            selb = pers.tile([128, 256], FP8, tag="selb")
            nc.sync.dma_start(selb[:], sel[:])
            eyeb = pers.tile([128, 128], BF16, tag="eyeb")
            nc.sync.dma_start(eyeb[:], eye[:])
            mskb = pers.tile([128, 2 * CH2], BF16, tag="mskb")
            nc.sync.dma_start(mskb[:, 0:CH2], msk[0, :, :])
            nc.sync.dma_start(mskb[:, CH2:2 * CH2], msk[1, :, :])
            rtb = pers.tile([16, 16], BF16, tag="rtb")
            nc.sync.dma_start(rtb[:], rt[:])
            cmb = pers.tile([128, 32], F32, tag="cmb")
            nc.sync.dma_start(cmb[:, 0:16], cm[0, :, :])
            nc.sync.dma_start(cmb[:, 16:32], cm[1, :, :])
            ones1 = pers.tile([128, 1], BF16, tag="ones1")
            nc.vector.memset(ones1[:], 1.0)
            onesf = pers.tile([128, 1], F32, tag="onesf")
            nc.vector.memset(onesf[:], 1.0)
            negf = pers.tile([128, 1], F32, tag="negf")
            nc.vector.memset(negf[:], -2.0)
            decb = pers.tile([128, 1], F32, tag="decb")
            nc.vector.memset(decb[:], DEC_BIAS)
            wub = pers.tile([128, 2], F32, tag="wub")
            nc.vector.memset(wub[:], 1.0)
            nc.sync.dma_start(wu_in[:], wub[:])
            nc.gpsimd.collective_compute(
                "ReduceScatter", ALU.add, replica_groups=grp,
                ins=[wu_in[:].opt()], outs=[wu_out[:].opt()])

            # ---- squares (ACT 5 / DVE 3) + SEL ssq DR-matmuls ----
            selv = selb[:].rearrange("p (two j) -> p two j", two=2)
            ssq_q = [pps.tile([128, 512], F32, tag="ps", name=f"ssq{j}")
                     for j in range(4)]
            for k in range(KT):
                sq = psq.tile([128, 2 * R], FP8, tag="sq")
                src = xb[k][:]
                if k % 2 == 1:
                    nc.vector.tensor_tensor(sq[:], src, src, ALU.mult)
                else:
                    nc.scalar.activation(sq[:], src, AF.Square)
                sqv = sq[:].rearrange("p (two r) -> p two r", two=2)
                for j in range(4):
                    nc.tensor.matmul(ssq_q[j][:], selv,
                                     sqv[:, :, 512 * j:512 * (j + 1)],
                                     start=(k == 0), stop=(k == KT - 1),
                                     perf_mode=PM.DoubleRow)

            # scale_t = S/sqrt(128*ssq) fp8 [128, R], pre-replicated by the
            # wide SEL (SEL[p,s,j] = (p%16 == j%16))
            scale_t = pers.tile([128, R], FP8, tag="scale_t")
            for j in range(4):
                nc.scalar.activation(scale_t[:, 512 * j:512 * (j + 1)],
                                     ssq_q[j][:], AF.Abs_reciprocal_sqrt,
                                     scale=128.0 / (S * S))

            # ---- normalize in place, DVE only (k-order) ----
            for k in range(KT):
                for s in range(2):
                    sl = xb[k][:, s * R:(s + 1) * R]
                    nc.vector.tensor_tensor(sl, sl, scale_t[:], ALU.mult)

            # ---- gram, k-outer passes over 8 rotating [128,512] psums ----
            def vk(k):
                return xb[k][:].rearrange("p (two r) -> p two r", two=2)

            def run_pass(units, cast_w):
                """units: list of (dst_dram, dst_row, dst_col, pieces) where
                pieces = [(psum_off, sim_col, w, tile_row)]; one psum/unit."""
                tiles = []
                for ui, (dst, drow, dcol, pieces) in enumerate(units):
                    pt = pps.tile([128, cast_w], F32, tag="ps",
                                  name=f"g{ui}")
                    tiles.append(pt)
                for k in range(KT):
                    v = vk(k)
                    for pt, (dst, drow, dcol, pieces) in zip(tiles, units):
                        for po, sc, w, tr in pieces:
                            lhsT = v[:, :, 128 * tr:128 * (tr + 1)]
                            nc.tensor.matmul(pt[:, po:po + w], lhsT,
                                             v[:, :, sc:sc + w],
                                             start=(k == 0), stop=(k == KT - 1),
                                             perf_mode=PM.DoubleRow)
                for pt, (dst, drow, dcol, pieces) in zip(tiles, units):
                    sb = psim.tile([128, cast_w], F8E5, tag="simsb")
                    nc.vector.tensor_copy(sb[:], pt[:])
                    nc.sync.dma_start(
                        dst[drow:drow + 64, dcol:dcol + cast_w], sb[0:64, :])
                    nc.sync.dma_start(
                        dst[drow + 64:drow + 128, dcol:dcol + cast_w],
                        sb[64:128, :])

            # phase A: ch1 (tile-rows 0..7), two 512-col passes
            for half in range(2):
                units = []
                for t in range(8):
                    co = 512 * half
                    pieces = [(0, 128 * t + co, 512, t)]
                    units.append((cc1_in, 128 * t, co, pieces))
                run_pass(units, 512)

            nc.gpsimd.collective_compute(
                "ReduceScatter", ALU.add, replica_groups=grp,
                ins=[cc1_in[:].opt()], outs=[cc1_out[:].opt()])

            # phase B: ch2, col passes [0:512),[512:1024),[1024:1152)
            for lo, hi in ((0, 512), (512, 1024), (1024, CH2)):
                units = []
                for c in range(8):
                    units.append((cc2_in, 128 * c, lo, _pieces(c, lo, hi)))
                run_pass(units, hi - lo)

            # ---- ch1 loss work (overlaps RS2) ----
            simr1 = plo.tile([128, CH1], F8E5, tag="simr1")
            nc.sync.dma_start(simr1[0:64, :], cc1_out[0:64, :])
            nc.sync.dma_start(simr1[64:128, :], cc1_out[64:128, :])
            ex1 = plo.tile([128, CH1], BF16, tag="ex1")
            rsA1 = psm.tile([128, 1], F32, tag="rsA1")
            nc.scalar.activation(ex1[:], simr1[:], AF.Exp, scale=INV_T_S2,
                                 accum_out=rsA1[:])
            scr3 = plo.tile([128, 128], BF16, tag="scr3")
            expdA = psm.tile([128, 1], F32, tag="expdA")
            nc.vector.scalar_tensor_tensor(
                scr3[:], ex1[:, 0:128], 1.0, eyeb[:], ALU.mult, ALU.mult,
                accum_out=expdA[:])
            ps_col = pps.tile([128, 16], F32, tag="ps")
            for j in range(1, 8):
                nc.tensor.matmul(ps_col[:, j - 1:j],
                                 ex1[:, 128 * j:128 * (j + 1)], ones1[:],
                                 start=True, stop=True)

            nc.gpsimd.collective_compute(
                "ReduceScatter", ALU.add, replica_groups=grp,
                ins=[cc2_in[:].opt()], outs=[cc2_out[:].opt()])

            # ---- ch2 loss work ----
            simr2 = plo.tile([128, CH2], F8E5, tag="simr2")
            nc.sync.dma_start(simr2[0:64, :], cc2_out[0:64, :])
            nc.sync.dma_start(simr2[64:128, :], cc2_out[64:128, :])
            ex2 = plo.tile([128, CH2], BF16, tag="ex2")
            rs2t = psm.tile([128, 1], F32, tag="rs2t")
            nc.scalar.activation(ex2[:], simr2[:], AF.Exp, scale=INV_T_S2,
                                 accum_out=rs2t[:])

            scr = plo.tile([128, CH2], BF16, tag="scr")
            rsA2 = psm.tile([128, 1], F32, tag="rsA2")
            nc.vector.scalar_tensor_tensor(
                scr[:], ex2[:], 1.0, mskb[:, 0:CH2], ALU.mult, ALU.mult,
                accum_out=rsA2[:])
            scr2 = plo.tile([128, CH2], BF16, tag="scr")
            expdB = psm.tile([128, 1], F32, tag="expdB")
            nc.vector.scalar_tensor_tensor(
                scr2[:], ex2[:], 1.0, mskb[:, CH2:2 * CH2], ALU.mult, ALU.mult,
                accum_out=expdB[:])
            # positives: ln of exp-diag of pos tile (ex2 chunk 0)
            scr4 = plo.tile([128, 128], BF16, tag="scr3")
            pde = psm.tile([128, 1], F32, tag="pde")
            nc.vector.scalar_tensor_tensor(
                scr4[:], ex2[:, 0:128], 1.0, eyeb[:], ALU.mult, ALU.mult,
                accum_out=pde[:])
            lnpd = psm.tile([128, 1], F32, tag="lnpd")
            nc.scalar.activation(lnpd[:], pde[:], AF.Ln)

            denA = psm.tile([128, 1], F32, tag="denA")
            nc.vector.tensor_tensor(denA[:], rsA1[:], rsA2[:], ALU.add)
            nc.vector.tensor_sub(denA[:], denA[:], expdA[:])
            denB = psm.tile([128, 1], F32, tag="denB")
            nc.vector.tensor_sub(denB[:], rs2t[:], rsA2[:])
            nc.vector.tensor_sub(denB[:], denB[:], expdB[:])

            for j in range(9):
                nc.tensor.matmul(ps_col[:, 7 + j:8 + j],
                                 ex2[:, 128 * j:128 * (j + 1)], ones1[:],
                                 start=True, stop=True)
            pc_sb = psm.tile([128, 16], BF16, tag="pc_sb")
            nc.vector.tensor_copy(pc_sb[:], ps_col[:])
            ps_t = pps.tile([128, 128], BF16, tag="ps")
            nc.tensor.transpose(ps_t[0:16, :], pc_sb[:], eyeb[:])
            pt_sb = psm.tile([16, 128], BF16, tag="pt_sb")
            nc.vector.tensor_copy(pt_sb[:], ps_t[0:16, :])
            ps_add = pps.tile([128, 16], F32, tag="ps")
            nc.tensor.matmul(ps_add[:], pt_sb[:], rtb[:], start=True, stop=True)

            den16 = psm.tile([128, 16], F32, tag="den16")
            nc.vector.scalar_tensor_tensor(
                den16[:], cmb[:, 0:16], 1.0,
                denA[:].to_broadcast((128, 16)), ALU.mult, ALU.mult)
            t2 = psm.tile([128, 16], F32, tag="t2")
            nc.vector.scalar_tensor_tensor(
                t2[:], cmb[:, 16:32], 1.0,
                denB[:].to_broadcast((128, 16)), ALU.mult, ALU.mult)
            nc.vector.tensor_tensor(den16[:], den16[:], t2[:], ALU.add)
            nc.vector.tensor_tensor(den16[:], den16[:], ps_add[:], ALU.add)

            nc.sync.dma_start(ccd_in[:], den16[:])
            nc.gpsimd.collective_compute(
                "ReduceScatter", ALU.add, replica_groups=grp,
                ins=[ccd_in[:].opt()], outs=[ccd_out[:].opt()])

            den_sb = psm.tile([16, 16], F32, tag="den_sb")
            nc.sync.dma_start(den_sb[:], ccd_out[:])
            lnj = psm.tile([16, 16], F32, tag="lnj")
            lnacc = psm.tile([16, 1], F32, tag="lnacc")
            nc.scalar.activation(lnj[:], den_sb[:], AF.Ln, accum_out=lnacc[:])

            loss_ps = pps.tile([1, 1], F32, tag="ps")
            nc.tensor.matmul(loss_ps[:], lnacc[:], onesf[0:16, :],
                             start=True, stop=False)
            nc.tensor.matmul(loss_ps[:], lnpd[:], negf[:],
                             start=False, stop=True)
            out_sb = pers.tile([1, 1], F32, tag="outsb")
            nc.vector.tensor_copy(out_sb[:], loss_ps[:])
            nc.sync.dma_start(y[:], out_sb[:])

    nc.compile()
    _CACHE["nc"] = nc
    return nc


def _make_inputs(emb_i, emb_j):
    e = np.concatenate([np.asarray(emb_i, np.float32),
                        np.asarray(emb_j, np.float32)], axis=0)
    sel = np.zeros((128, 2, 128), np.float32)
    for p in range(128):
        sel[p, :, np.arange(p % 16, 128, 16)] = 1.0
    sel = sel.reshape(128, 256).astype(ml_dtypes.float8_e4m3)
    eye = np.eye(128, dtype=np.float32).astype(ml_dtypes.bfloat16)

    in_maps = []
    for c in range(NCORES):
        loc = e[:, :, 16 * c:16 * (c + 1)]
        t = loc.reshape(R, 8, 8, 2, 16)
        t = t.transpose(1, 2, 4, 3, 0)
        x = np.ascontiguousarray(t).reshape(KT, 128, 2 * R).astype(
            ml_dtypes.float8_e4m3)

        wa2 = 1024 - 128 * c
        msk = np.zeros((2, 128, CH2), np.float32)
        msk[0, :, 0:wa2] = 1.0
        jd = 8 - c
        msk[1, np.arange(128), 128 * jd + np.arange(128)] = 1.0

        rt_m = np.zeros((16, 16), np.float32)
        for col in range(16):
            if col < 15 - c:
                rt_m[col, c + 1 + col] = 1.0
            elif col >= 16 - c:
                rt_m[col, col] = 1.0
        cm_m = np.zeros((2, 128, 16), np.float32)
        cm_m[0, :, c] = 1.0
        cm_m[1, :, 15 - c] = 1.0

        in_maps.append({
            "x": x, "sel": sel, "eye": eye,
            "msk": msk.astype(ml_dtypes.bfloat16),
            "rt": rt_m.astype(ml_dtypes.bfloat16),
            "cm": cm_m.astype(np.float32),
        })
    return in_maps


def run(emb_i, emb_j, **spmd_kwargs):
    nc = _build_nc()
    in_maps = _make_inputs(emb_i, emb_j)
    res = bass_utils.run_bass_kernel_spmd(
        nc, in_maps, core_ids=list(range(NCORES)), **spmd_kwargs)
    total = sum(float(r["y"][0, 0]) for r in res.results)
    return np.array(total / R, dtype=np.float32), res


def kernel(emb_i, emb_j):
    loss, _ = run(emb_i, emb_j)
    return loss

